# revision 9
# baseline (speedup 1.0000x reference)
"""ConvLSTM (T=16, 4->64 hidden, 3x3 conv, 256x256) on 8 Trainium2 NeuronCores.

Strategy: spatial sharding, 32 image rows per core, ZERO inter-core
communication.  Each core redundantly computes a shrinking halo stripe
(62 rows at t=0 down to 32 rows at t=15), so after 16 steps its owned
32 rows are exact.  Out-of-image rows on edge cores are forced to h=c=0
by a mask input channel with a -30000 weight (sigmoid underflows to 0).

Conv 3x3 (68->256 ch) per step as matmuls (bf16 x bf16 -> fp32 PSUM):
  - h kept in SBUF as h2[128, rows, 258]: partitions 0-63 = h(y),
    partitions 64-127 = h(y+1) (shifted copy).  Pairing taps (dy=-1,dx)
    with (dy=0,dx) gives 3 full-K=128 matmuls covering 6 taps.
  - remaining taps (1,-1),(1,0) run as two concurrent K=64 row-tiled
    matmuls (array rows 0-63 / 64-127); tap (1,1) (K=64, rows 64-127)
    runs concurrently with the whole x-contribution (K=37 im2col'd on
    host, rows 0-36).
LSTM gate math: gates split into PSUM halves A=[i;f], B=[g;o]; one
sigmoid per half (tanh(g) = 2*sigmoid(2g)-1 with the 2x folded into
W_g/b_g; cell state stored as c/2 so the fixup is a single fused
(x-0.5)*y op); tanh(c) via ACT scale=2.
"""

import os
import sys
import types

sys.path.insert(0, "/opt/trn_rl_repo")

import numpy as np
import ml_dtypes

BF16 = ml_dtypes.bfloat16

# ---------------------------------------------------------------- constants
CORES = 8
T_FULL = 16
HID = 64
IN_CH = 4
IMG = 256
OWN = IMG // CORES          # 32 rows owned per core
LROWS = 2 * OWN             # 64-row local window (owned rows at [16,48))
WPAD = IMG + 2              # 258
KX = 4 * 9 + 1              # 37 = im2col x channels + mask plane
MASK_W = 30000.0


def _install_ntff_hook():
    """bass_utils trace=True needs antenv.axon_hooks; shim it from trn_boot."""
    if "antenv.axon_hooks" in sys.modules:
        return
    try:
        import trn_agent_boot.trn_boot as tb
        import antenv

        hook = tb._ntff_profile_via_ctypes("/opt/axon/libaxon_pjrt.so")
        mod = types.ModuleType("antenv.axon_hooks")
        mod.get_axon_ntff_profile_hook = lambda: hook
        mod.set_axon_ntff_profile_hook = lambda h: None
        sys.modules["antenv.axon_hooks"] = mod
        antenv.axon_hooks = mod
    except Exception:
        pass


# ---------------------------------------------------------------- device IR
def build_nc(T=T_FULL, dbg=False):
    import concourse.bass as bass  # noqa: F401
    import concourse.mybir as mybir
    import concourse.tile as tile
    from concourse import bacc

    f32 = mybir.dt.float32
    bf16 = mybir.dt.bfloat16
    Alu = mybir.AluOpType
    Act = mybir.ActivationFunctionType

    nc = bacc.Bacc(None, target_bir_lowering=False)

    xcol_d = nc.declare_dram_parameter("xcol", [T, KX, LROWS, IMG], bf16, isOutput=False)
    wts_d = nc.declare_dram_parameter("wts", [128, 5, 256], bf16, isOutput=False)
    bvec_d = nc.declare_dram_parameter("bvec", [128, 3], f32, isOutput=False)
    wout_d = nc.declare_dram_parameter("wout", [HID, 1], bf16, isOutput=False)
    out_d = nc.declare_dram_parameter("out", [OWN, IMG], f32, isOutput=True)
    if dbg:
        hdbg_d = nc.declare_dram_parameter("hdbg", [128, LROWS, WPAD], bf16, isOutput=True)
        cdbg_d = nc.declare_dram_parameter("cdbg", [128, (LROWS - 2) * IMG], f32, isOutput=True)

    CSTRIDE = IMG  # c state row stride (no width pad)

    with tile.TileContext(nc) as tc:
        with (
            tc.tile_pool(name="persist", bufs=1) as pp,
            tc.tile_pool(name="tmps", bufs=3) as tp,
            tc.tile_pool(name="psum", bufs=2, space="PSUM") as qp,
        ):
            h2 = pp.tile([128, LROWS, WPAD], bf16, tag="h2")
            c_st = pp.tile([128, (LROWS - 2) * CSTRIDE], f32, tag="c")
            xcol = pp.tile([KX, 2, LROWS, IMG], bf16, tag="xcol")
            wts = pp.tile([128, 5, 256], bf16, tag="wts")
            bvec = pp.tile([128, 3], f32, tag="bvec")
            wout = pp.tile([HID, 1], bf16, tag="wout")

            wts3b_lo = pp.tile([64, 1, 256], bf16, tag="w3lo")
            wts4b_lo = pp.tile([64, 1, 256], bf16, tag="w4lo")
            nc.sync.dma_start(wts3b_lo[:, 0], wts_d[64:128, 3])
            nc.sync.dma_start(wts4b_lo[:, 0], wts_d[64:128, 4])
            nc.sync.dma_start(wts[:], wts_d[:])
            nc.sync.dma_start(bvec[:], bvec_d[:])
            nc.sync.dma_start(wout[:], wout_d[:])
            nc.vector.memset(h2[:], 0.0)
            nc.gpsimd.memset(c_st[:], 0.0)
            nc.sync.dma_start(xcol[:, 0], xcol_d[0])

            def cs(a, nfree):
                """c-state slice [64-127, rows a..] of nfree elements."""
                o = (a - 1) * CSTRIDE
                return c_st[64:128, o : o + nfree]

            pend = None

            def flush(p):
                ht, pa, pnr = p
                nfp = IMG * pnr
                nc.sync.dma_start(
                    h2[0:64, pa : pa + pnr, 1:257], ht[:, :nfp])
                nc.sync.dma_start(
                    h2[64:128, pa - 1 : pa - 1 + pnr, 1:257], ht[:, :nfp])

            for t in range(T):
                if t > 0 and pend is not None:
                    flush(pend)
                    pend = None
                buf = t % 2
                if t + 1 < T:
                    nc.sync.dma_start(xcol[:, (t + 1) % 2], xcol_d[t + 1])
                lo, hi = 1 + t, LROWS - 1 - t
                npairs = (hi - lo) // 2
                for g0 in range(0, npairs, 2):
                    gp = min(2, npairs - g0)
                    a = lo + 2 * g0      # first output row of this group
                    nr = 2 * gp          # rows in group
                    nf = IMG * nr        # free elements
                    psA = qp.tile([128, 1024], f32, tag="psA", name="psA")[:, :nf]
                    psB = qp.tile([128, 1024], f32, tag="psB", name="psB")[:, :nf]
                    for p in range(gp):
                        l = a + 2 * p
                        for half, ps in ((0, psA), (1, psB)):
                            sl = ps[:, 512 * p : 512 * p + 512]
                            mc = slice(128 * half, 128 * half + 128)
                            if t == 0:
                                # h==0: only the x/mask contribution
                                nc.tensor.matmul(
                                    sl, wts[0:KX, 4, mc], xcol[:, buf, l : l + 2, :],
                                    start=True, stop=True)
                                continue
                            # slots 0-2: taps (-1,dx)+(0,dx), K=128
                            for s, dx in ((0, -1), (1, 0), (2, 1)):
                                nc.tensor.matmul(
                                    sl, wts[:, s, mc],
                                    h2[:, l - 1 : l + 1, 1 + dx : 257 + dx],
                                    start=(s == 0), stop=False)
                            # slot3a: tap (1,-1) via lower rows l+1,l+2
                            nc.tensor.matmul(
                                sl, wts[0:64, 3, mc],
                                h2[0:64, l + 1 : l + 3, 0:256],
                                start=False, stop=False)
                            # slot3b: tap (1,0) via upper rows l,l+1 (concurrent)
                            if os.environ.get("B64") != "1":
                                nc.tensor.matmul(
                                    sl, wts[64:128, 3, mc].at_base_partition(0) if hasattr(wts[64:128, 3, mc], 'at_base_partition') else wts[64:128, 3, mc],
                                    h2[64:128, l : l + 2, 1:257],
                                    start=False, stop=False) if False else None
                                nc.tensor.matmul(
                                    sl, wts3b_lo[:, 0, mc],
                                    h2[0:64, l + 1 : l + 3, 1:257],
                                    start=False, stop=False)
                            else:
                                nc.tensor.matmul(
                                    sl, wts[64:128, 3, mc],
                                    h2[64:128, l : l + 2, 1:257],
                                    start=False, stop=False)
                            # slot4a: x im2col + mask, K=37 (rows 0-36)
                            nc.tensor.matmul(
                                sl, wts[0:KX, 4, mc],
                                xcol[:, buf, l : l + 2, :],
                                start=False, stop=False)
                            # slot4b: tap (1,1) via upper, col +1 (concurrent)
                            if os.environ.get("B64") != "1":
                                nc.tensor.matmul(
                                    sl, wts4b_lo[:, 0, mc],
                                    h2[0:64, l + 1 : l + 3, 2:258],
                                    start=False, stop=True)
                            else:
                                nc.tensor.matmul(
                                    sl, wts[64:128, 4, mc],
                                    h2[64:128, l : l + 2, 2:258],
                                    start=False, stop=True)
                    # ---- LSTM pointwise for rows [a, a+nr) ----
                    sigA = tp.tile([128, 1024], f32, tag="sigA", name="sigA")[:, :nf]
                    sigB = tp.tile([128, 1024], f32, tag="sigB", name="sigB")[:, :nf]
                    tig = tp.tile([128, 1024], f32, tag="tig", name="tig")[64:128, :nf]
                    nc.scalar.activation(sigA, psA, Act.Sigmoid, bias=bvec[:, 0:1])
                    nc.scalar.activation(sigB, psB, Act.Sigmoid, bias=bvec[:, 1:2])
                    csl = cs(a, nf)
                    # tig = (sigmoid(2g) - 0.5) * i_s   [= i_s * tanh(g)/2]
                    nc.vector.scalar_tensor_tensor(
                        tig, sigB[0:64], 0.5, sigA[0:64], Alu.subtract, Alu.mult)
                    # c~ *= f_s
                    nc.vector.tensor_tensor(csl, csl, sigA[64:128], Alu.mult)
                    # c~ += tig
                    nc.vector.tensor_tensor(csl, tig, csl, Alu.add)
                    # tanh(2*c~) -> sigA[64:128] (f_s slot, dead now)
                    nc.scalar.activation(sigA[64:128], csl, Act.Tanh, scale=2.0)
                    # h = o_s * tanh_c  -> staged tmp (h2 write deferred:
                    # the NEXT group's dy=-1 taps must read the old row a-1)
                    htmp = tp.tile([128, 1024], bf16, tag="htmp", name="htmp")[64:128, :nf]
                    nc.vector.tensor_tensor(htmp, sigB[64:128], sigA[64:128], Alu.mult)
                    if pend is not None:
                        flush(pend)
                    pend = (htmp, a, nr)

            if pend is not None:
                flush(pend)
                pend = None

            # ---- final 1x1 conv over owned rows [16, 48) ----
            for j in range(OWN // 2):
                l = OWN // 2 + 2 * j
                pso = qp.tile([1, 512], f32, tag="psA", name="pso")
                nc.tensor.matmul(
                    pso[:], wout[0:HID, 0:1], h2[0:HID, l : l + 2, 1:257],
                    start=True, stop=True)
                o_t = tp.tile([1, 512], f32, tag="sigB", name="o_t")
                nc.scalar.activation(o_t[:], pso[:], Act.Identity, bias=bvec[0:1, 2:3])
                nc.sync.dma_start(out_d[2 * j : 2 * j + 2, :], o_t[:])
            if dbg:
                nc.sync.dma_start(hdbg_d[:], h2[:])
                nc.sync.dma_start(cdbg_d[:], c_st[:])

    nc.finalize()
    return nc


# ---------------------------------------------------------------- host prep
def pack_inputs(seq, W_conv, b_conv, W_out, b_out, T=T_FULL):
    seq = np.asarray(seq, np.float32)[:T]
    W_conv = np.asarray(W_conv, np.float32)
    b_conv = np.asarray(b_conv, np.float32)
    W_out = np.asarray(W_out, np.float32)
    b_out = np.asarray(b_out, np.float32)

    # gate reorder: A half = [i(0:64); f(64:128)], B half = [g; o]
    co_order = np.concatenate(
        [np.arange(0, 128), np.arange(192, 256), np.arange(128, 192)])
    Wr = W_conv[co_order]
    br = b_conv[co_order].copy()
    gsc = np.ones(256, np.float32)
    gsc[128:192] = 2.0       # fold tanh(g)=2*sigmoid(2g)-1 scale into W_g, b_g
    Wr = Wr * gsc[:, None, None, None]
    br = br * gsc

    wts = np.zeros((128, 5, 256), np.float32)
    for s, dx in ((0, -1), (1, 0), (2, 1)):
        wts[0:64, s, :] = Wr[:, 4:, 0, dx + 1].T
        wts[64:128, s, :] = Wr[:, 4:, 1, dx + 1].T
    wts[0:64, 3, :] = Wr[:, 4:, 2, 0].T
    wts[64:128, 3, :] = Wr[:, 4:, 2, 1].T
    for tap in range(9):
        dy, dx = tap // 3 - 1, tap % 3 - 1
        wts[4 * tap : 4 * tap + 4, 4, :] = Wr[:, 0:4, dy + 1, dx + 1].T
    wts[36, 4, :] = MASK_W
    wts[64:128, 4, :] = Wr[:, 4:, 2, 2].T
    wts = wts.astype(BF16)

    bvec = np.zeros((128, 3), np.float32)
    bvec[:, 0] = br[0:128]
    bvec[:, 1] = br[128:256]
    bvec[0, 2] = float(b_out[0])
    wout = W_out[0, :, 0, 0].reshape(HID, 1).astype(BF16)

    # padded input: height +-17, width +-1
    SP = np.zeros((T, IN_CH, IMG + 34, IMG + 2), np.float32)
    SP[:, :, 17 : 17 + IMG, 1 : 1 + IMG] = seq

    in_maps = []
    for i in range(CORES):
        xc = np.zeros((T, KX, LROWS, IMG), BF16)
        for tap in range(9):
            dy, dx = tap // 3 - 1, tap % 3 - 1
            r0 = 32 * i + 1 + dy
            xc[:, 4 * tap : 4 * tap + 4, :, :] = SP[
                :, :, r0 : r0 + LROWS, 1 + dx : 257 + dx]
        y = 32 * i - 16 + np.arange(LROWS)
        m = ((y < 0) | (y >= IMG)).astype(np.float32) * -1.0
        xc[:, 36, :, :] = m[None, :, None]
        in_maps.append({
            "xcol": np.ascontiguousarray(xc),
            "wts": wts,
            "bvec": bvec,
            "wout": wout,
        })
    return in_maps


# ---------------------------------------------------------------- entry
_CACHE = {}


def run(inputs, T=T_FULL, trace=False):
    _install_ntff_hook()
    from concourse.bass_utils import run_bass_kernel_spmd

    if T not in _CACHE:
        _CACHE[T] = build_nc(T)
    nc = _CACHE[T]
    in_maps = pack_inputs(
        inputs["seq"], inputs["W_conv"], inputs["b_conv"],
        inputs["W_out"], inputs["b_out"], T=T)
    res = run_bass_kernel_spmd(nc, in_maps, core_ids=list(range(CORES)), trace=trace)
    full = np.zeros((1, 1, IMG, IMG), np.float32)
    for i in range(CORES):
        full[0, 0, 32 * i : 32 * i + 32, :] = res.results[i]["out"]
    return full, res


def kernel(**inputs):
    full, _ = run(inputs, T=T_FULL, trace=False)
    return full
